# revision 8
# baseline (speedup 1.0000x reference)
"""HPWL (half-perimeter wirelength) via per-net segment max/min on 8 trn2 NeuronCores.

kernel(pos, pin2net_map, net_mask) -> float32[1]

Algorithm
---------
Per-net max/min are recovered from per-net *sums* of exponentials
(softmax trick):  max_i x_i = ln(sum_i e^{s*x_i})/s + O(ln k / s) where k is
the number of near-max ties; with s = 0.16 the systematic overestimate is
~4.3 * ln(k) length units per net (avg < 0.2), i.e. ~0.05% of the mean span —
far inside the 2e-2 relative tolerance.  This turns the segment max/min into
a segment SUM, which maps onto the TensorEngine: for a tile of 128 pins, a
one-hot(net mod 128) stationary matrix times the [128 pins x 4 payload]
moving matrix accumulates the per-net exponential sums of
(e^{+s(x-500)}, e^{-s(x-500)}, e^{+s(y-500)}, e^{-s(y-500)}) in PSUM.

Sharding: nets are range-partitioned across the 8 cores; the host routes
each pin (with its x, y, net%128 payload) to the core that owns its net and
groups the core's pins by 128-net block (net>>7), padding each block to a
fixed number of 128-pin tiles.  Pad pins carry net-col -1, whose one-hot
column is all-zero, so they contribute nothing.  All per-net work (the
exponential payloads, the one-hot grouping, the segment sums, the
log-recovery of max/min, empty-net/mask filtering and the final reduction)
happens on-device; the host only permutes/pads inputs and sums the 8
per-core partials.

exp and log use DVE bit tricks (IEEE exponent arithmetic with a quadratic
mantissa correction), so no ACT tables are involved.
"""

import numpy as np
import ml_dtypes

P = 128
NUM_PINS = 20_000_000
NUM_NETS = 5_000_000
N_CORES = 8
NETS_PER_CORE = NUM_NETS // N_CORES          # 625_000
S_SCALE = 0.16                                # exp scale per length unit
LOG2E_S = S_SCALE * 128.0 / float(np.log(2.0))   # bf16-bit units per length unit
BITS_BIAS = 127.0 * 128.0                     # bf16 exponent bias in bit units
X_CENTER = 500.0

_CACHE = {}


def _build_program(blocks, tiles_per_block, group_blocks):
    """Build the per-core Bass program.  Returns (nc, cols, blocks_padded)."""
    import concourse.bacc as bacc
    import concourse.mybir as mybir
    import concourse.tile as tile
    from concourse.bass import ds

    dt = mybir.dt
    blocks_padded = ((blocks + group_blocks - 1) // group_blocks) * group_blocks
    groups = blocks_padded // group_blocks
    cols = blocks_padded * tiles_per_block          # 128-pin tile columns
    gcols = group_blocks * tiles_per_block          # tile columns per group

    nc = bacc.Bacc(name=f"hpwl_{blocks_padded}_{tiles_per_block}")
    x_in = nc.dram_tensor("x_in", [P, cols], dt.float32, kind="ExternalInput")
    y_in = nc.dram_tensor("y_in", [P, cols], dt.float32, kind="ExternalInput")
    nm_in = nc.dram_tensor("nm_in", [P, cols], dt.bfloat16, kind="ExternalInput")
    mask_in = nc.dram_tensor("mask_in", [P, blocks_padded], dt.float32,
                             kind="ExternalInput")
    iota_in = nc.dram_tensor("iota_in", [P, 1, P], dt.bfloat16, kind="ExternalInput")
    out = nc.dram_tensor("out", [P, 1], dt.float32, kind="ExternalOutput")

    A = LOG2E_S                      # bf16-bit units per x unit
    BP = BITS_BIAS - X_CENTER * A    # z16 = x*A + BP  (plus stream)
    BM = BITS_BIAS + X_CENTER * A    # z16 = -x*A + BM (minus stream)

    with tile.TileContext(nc) as tc:
        with tc.tile_pool(name="persist", bufs=1) as pp:
            bins = pp.tile([P, blocks_padded, 4], dt.float32)
            iota = pp.tile([P, 1, P], dt.bfloat16)
            nc.vector.memset(bins[:], 0)
            nc.sync.dma_start(iota[:], iota_in.ap()[:])

            with tc.tile_pool(name="stream", bufs=2) as sp, \
                 tc.tile_pool(name="psum", bufs=8, space="PSUM") as psp, \
                 tc.For_i(0, groups) as g:
                xg = sp.tile([P, gcols], dt.float32, tag="xg")
                yg = sp.tile([P, gcols], dt.float32, tag="yg")
                ng = sp.tile([P, gcols], dt.bfloat16, tag="ng")
                nc.sync.dma_start(xg[:], x_in.ap()[:, ds(g * gcols, gcols)])
                nc.sync.dma_start(yg[:], y_in.ap()[:, ds(g * gcols, gcols)])
                nc.sync.dma_start(ng[:], nm_in.ap()[:, ds(g * gcols, gcols)])

                # 4 exp payload streams as bf16 bit patterns, built by int16
                # affine convert:  bits = round(x*A + B)  ==  bf16(e^{s(x-c)})
                pl = sp.tile([P, gcols, 4], dt.bfloat16, tag="pl")
                pli = pl[:].bitcast(dt.int16)
                nc.vector.tensor_scalar(pli[:, :, 0], xg[:], A, BP,
                                        mybir.AluOpType.mult, mybir.AluOpType.add)
                nc.vector.tensor_scalar(pli[:, :, 1], xg[:], -A, BM,
                                        mybir.AluOpType.mult, mybir.AluOpType.add)
                nc.vector.tensor_scalar(pli[:, :, 2], yg[:], A, BP,
                                        mybir.AluOpType.mult, mybir.AluOpType.add)
                nc.vector.tensor_scalar(pli[:, :, 3], yg[:], -A, BM,
                                        mybir.AluOpType.mult, mybir.AluOpType.add)

                # one-hot over net-col: oh[p, t, c] = (ng[p, t] == c)
                oh = sp.tile([P, gcols, P], dt.bfloat16, tag="oh")
                nc.vector.tensor_tensor(
                    oh[:],
                    ng[:].rearrange("p (t u) -> p t u", u=1).to_broadcast(
                        [P, gcols, P]),
                    iota[:].to_broadcast([P, gcols, P]),
                    mybir.AluOpType.is_equal)

                for b2 in range(group_blocks):
                    ps = psp.tile([P, 4], dt.float32, space="PSUM", tag="ps")
                    for t in range(tiles_per_block):
                        k = b2 * tiles_per_block + t
                        nc.tensor.matmul(ps[:],
                                         oh[:, k, :], pl[:, k, :],
                                         start=(t == 0),
                                         stop=(t == tiles_per_block - 1))
                    nc.scalar.copy(
                        bins[:, ds(g * group_blocks + b2, 1), :]
                        .rearrange("p u v -> p (u v)"),
                        ps[:])

            fpool = tc.tile_pool(name="final", bufs=1)
            fp = fpool.__enter__()
            # ---- final reduction ----
            # Per stream v: L = log2(bins[:, :, v]) via the exponent bit
            # trick with quadratic mantissa correction; Lsum = sum of the 4
            # logs = span / (ln2/s).  Temps are [P, blocks_padded] to stay
            # inside SBUF at full scale.
            bp = blocks_padded
            lsum = fp.tile([P, bp], dt.float32, tag="lsum")
            for v in range(4):
                ui = fp.tile([P, bp], dt.int32, tag="ui")
                nc.vector.tensor_copy(
                    ui[:].rearrange("p (b u) -> p b u", u=1),
                    bins[:, :, v:v + 1].bitcast(dt.int32))
                lf = fp.tile([P, bp], dt.float32, tag="lf")
                nc.vector.tensor_copy(lf[:], ui[:])
                nc.vector.tensor_scalar(lf[:], lf[:], 1.0 / (1 << 23), -127.0,
                                        mybir.AluOpType.mult, mybir.AluOpType.add)
                fi = fp.tile([P, bp], dt.int32, tag="fi")
                nc.vector.tensor_scalar(fi[:], ui[:], (1 << 23) - 1, None,
                                        mybir.AluOpType.bitwise_and)
                fr = fp.tile([P, bp], dt.float32, tag="fr")
                nc.vector.tensor_copy(fr[:], fi[:])
                nc.vector.tensor_scalar(fr[:], fr[:], 1.0 / (1 << 23), None,
                                        mybir.AluOpType.mult)
                f2 = fi[:].bitcast(dt.float32)
                nc.vector.tensor_tensor(f2, fr[:], fr[:], mybir.AluOpType.mult)
                nc.vector.tensor_tensor(fr[:], fr[:], f2, mybir.AluOpType.subtract)
                nc.vector.tensor_scalar(fr[:], fr[:], 0.344, None,
                                        mybir.AluOpType.mult)
                nc.vector.tensor_tensor(lf[:], lf[:], fr[:], mybir.AluOpType.add)
                if v == 0:
                    nc.vector.tensor_copy(lsum[:], lf[:])
                else:
                    nc.vector.tensor_tensor(lsum[:], lsum[:], lf[:],
                                            mybir.AluOpType.add)

            # valid = (S+x > 0) * net_mask ; empty/padded nets have S == 0
            vld = fp.tile([P, bp], dt.float32, tag="ui")
            nc.vector.tensor_scalar(
                vld[:].rearrange("p (b u) -> p b u", u=1),
                bins[:, :, 0:1], 0.0, None, mybir.AluOpType.is_gt)
            msk = fp.tile([P, bp], dt.float32, tag="lf")
            nc.sync.dma_start(msk[:], mask_in.ap()[:])
            nc.vector.tensor_tensor(vld[:], vld[:], msk[:], mybir.AluOpType.mult)
            nc.vector.tensor_tensor(lsum[:], lsum[:], vld[:], mybir.AluOpType.mult)

            acc = fp.tile([P, 1], dt.float32, tag="acc")
            nc.vector.tensor_reduce(acc[:], lsum[:], mybir.AxisListType.X,
                                    mybir.AluOpType.add)
            nc.vector.tensor_scalar(acc[:], acc[:], float(np.log(2.0)) / S_SCALE,
                                    None, mybir.AluOpType.mult)
            nc.sync.dma_start(out.ap()[:], acc[:])
            fpool.__exit__(None, None, None)

    nc.compile()
    _install_waitsplit(nc)
    return nc, cols, blocks_padded


def _install_waitsplit(nc):
    """This walrus build rejects >1 sync-wait per instruction; hoist excess
    waits onto NoOps inserted just before, on the same engine."""
    import orjson
    raw = type(nc).to_json_bytes.__get__(nc)()
    j = orjson.loads(raw)
    for f in j["functions"]:
        for blk in f["blocks"]:
            outl = []
            for ins in blk["instructions"]:
                si = ins.get("sync_info")
                waits = (si or {}).get("on_wait") or []
                if len(waits) > 1:
                    for i, w in enumerate(waits[:-1]):
                        outl.append({
                            "debug": ins.get("debug", 0), "engine": ins["engine"],
                            "ins": [], "outs": [],
                            "name": f"{ins['name']}-ws{i}", "opcode": "NoOp",
                            "sync_info": {"on_update": [], "on_wait": [w]},
                            "text_hint": "waitsplit",
                        })
                    si["on_wait"] = waits[-1:]
                outl.append(ins)
            blk["instructions"] = outl
    data = orjson.dumps(j)
    nc.to_json_bytes = lambda: data


def _shard_inputs(pos, pin2net_map, net_mask, tiles_per_block=None):
    """Host sharding: route pins to net-range cores, group by 128-net block,
    pad blocks to whole 128-pin tiles.  Returns (in_maps, meta)."""
    n_pins = pin2net_map.shape[0]
    n_nets = net_mask.shape[0]
    npc = n_nets // N_CORES
    blocks = (npc + P - 1) // P

    x = np.ascontiguousarray(pos[:n_pins], dtype=np.float32)
    y = np.ascontiguousarray(pos[n_pins:], dtype=np.float32)
    net = np.ascontiguousarray(pin2net_map, dtype=np.int64)

    order = np.argsort(net, kind="stable")
    net_s = net[order]
    x_s = x[order]
    y_s = y[order]
    core_s = net_s // npc
    local_s = net_s - core_s * npc
    block_s = local_s >> 7
    col_s = local_s & (P - 1)

    # per (core, block) counts and tile capacity
    gb = core_s * blocks + block_s
    counts = np.bincount(gb, minlength=N_CORES * blocks)
    max_cnt = int(counts.max()) if counts.size else 0
    need_tpb = max(1, (max_cnt + P - 1) // P)
    if tiles_per_block is None:
        tiles_per_block = need_tpb
    else:
        tiles_per_block = max(tiles_per_block, need_tpb)

    group_blocks = 32
    blocks_padded = ((blocks + group_blocks - 1) // group_blocks) * group_blocks
    cols = blocks_padded * tiles_per_block
    cap = tiles_per_block * P

    # slot of each pin inside its (core, block) bucket
    starts = np.zeros(N_CORES * blocks, dtype=np.int64)
    np.cumsum(counts[:-1], out=starts[1:])
    rank = np.arange(n_pins, dtype=np.int64) - starts[gb]
    dest = block_s * cap + rank  # within-core flat slot

    in_maps = []
    iota_np = np.broadcast_to(
        np.arange(P, dtype=np.float32), (P, P)).astype(ml_dtypes.bfloat16)\
        .reshape(P, 1, P).copy()
    for c in range(N_CORES):
        sel = core_s == c
        d = dest[sel]
        total = blocks_padded * cap
        xa = np.full(total, X_CENTER, dtype=np.float32)
        ya = np.full(total, X_CENTER, dtype=np.float32)
        na = np.full(total, -1.0, dtype=np.float32)
        xa[d] = x_s[sel]
        ya[d] = y_s[sel]
        na[d] = col_s[sel].astype(np.float32)
        # slot s -> (partition s%128, tile-col s//128); SBUF [128, cols] is
        # partition-major from DRAM, so transpose the (cols, 128) grid.
        xa = xa.reshape(cols, P).T.copy()
        ya = ya.reshape(cols, P).T.copy()
        na = na.reshape(cols, P).T.astype(ml_dtypes.bfloat16)
        mk = np.zeros((P, blocks_padded), dtype=np.float32)
        base = c * npc
        nloc = np.arange(npc, dtype=np.int64)
        mkfull = np.zeros(blocks_padded * P, dtype=np.float32)
        mkfull[nloc] = net_mask[base:base + npc].astype(np.float32)
        mk = mkfull.reshape(blocks_padded, P).T.copy()
        in_maps.append({"x_in": xa, "y_in": ya, "nm_in": na,
                        "mask_in": mk, "iota_in": iota_np})
    meta = (blocks, tiles_per_block, group_blocks)
    return in_maps, meta


def _get_program(meta):
    key = meta
    if key not in _CACHE:
        blocks, tpb, gb = meta
        _CACHE[key] = _build_program(blocks, tpb, gb)
    return _CACHE[key]


def kernel(pos, pin2net_map, net_mask):
    from concourse import bass2jax
    in_maps, meta = _shard_inputs(np.asarray(pos), np.asarray(pin2net_map),
                                  np.asarray(net_mask))
    nc, cols, blocks_padded = _get_program(meta)
    res = bass2jax.run_bass_via_pjrt(nc, in_maps, n_cores=N_CORES)
    total = 0.0
    for r in res:
        total += float(np.asarray(r["out"], dtype=np.float64).sum())
    return np.asarray([total], dtype=np.float32)
